# revision 34
# baseline (speedup 1.0000x reference)
"""Trainium2 Bass kernel for nn_Diffuser (sparse_attention).

Key algebraic identity: the reference attention has NO softmax, so
    y_rh = (q k_rh^T s)(q k_rh^T s)^T v = s^2 * q (k_rh^T k_rh) (q^T v)
    mean_r y_rh = q @ Gbar_h @ (q_h^T v_h),
    Gbar_h = s^2/R * sum_r k_rh^T k_rh   (64x64, precomputed once).

Per step, per head: w = q^T v (64x64), P = Gbar w (64x64), z^T = P^T-as-lhsT
@ q^T.  The O(N^3) attention chain disappears entirely.

Sharding: pure data-parallel over B=8 across 8 NeuronCores (weights + ref
replicated, zero collectives).  All matmuls contract over the partition dim
(c-major layouts); f32r (TF32-like) full 128x128 tiles only — sub-tile shapes
are zero-padded (f32r rejects PE row/col tiling).
"""

import numpy as np
from contextlib import ExitStack

import concourse.bass as bass
import concourse.tile as tile
from concourse import bacc, mybir
from concourse.bass_utils import run_bass_kernel_spmd
from concourse.masks import make_identity
from concourse.tile_rust import add_dep_helper

F32 = mybir.dt.float32
F32R = mybir.dt.float32r
AF = mybir.ActivationFunctionType

D = 768
H = 12
HD = 64
R = 10
N = 256
STEPS = 3
NB = 8
CC = D // 128
SCALE = HD ** -0.5
EPS = 1e-5
GS = SCALE * SCALE / R  # folded into Gbar


def _emit(nc, tc, ctx, t_x, t_ref, t_Wqv, t_Wk, t_Wproj, t_bproj, t_gamma, t_beta, t_out,
          iters=1):
    const = ctx.enter_context(tc.tile_pool(name="const", bufs=1))
    persist = ctx.enter_context(tc.tile_pool(name="persist", bufs=1))

    ident = const.tile([128, 128], F32)
    make_identity(nc, ident)
    ones_f = const.tile([128, 128], F32)
    nc.vector.memset(ones_f, 1.0)
    eps_sb = const.tile([128, 1], F32)
    nc.vector.memset(eps_sb, EPS)
    ones128 = const.tile([128, 128], F32R)
    nc.scalar.copy(ones128[:], ones_f[:])
    zsrc = const.tile([128, N], F32)
    nc.vector.memset(zsrc, 0.0)

    gamma_sb = const.tile([128, CC], F32)
    beta_sb = const.tile([128, CC], F32)
    bproj_sb = const.tile([128, CC], F32)
    nc.sync.dma_start(out=gamma_sb, in_=t_gamma.ap().rearrange("(c p) -> p c", p=128))
    nc.sync.dma_start(out=beta_sb, in_=t_beta.ap().rearrange("(c p) -> p c", p=128))
    nc.sync.dma_start(out=bproj_sb, in_=t_bproj.ap().rearrange("(c p) -> p c", p=128))

    # ---- resident weights ----
    Wproj_r = []
    with tc.tile_pool(name="wload", bufs=3) as wload:
        for cc in range(CC):
            w = wload.tile([128, D], F32, tag="wproj")
            nc.sync.dma_start(out=w, in_=t_Wproj.ap()[cc * 128:(cc + 1) * 128, :])
            wr = persist.tile([128, D], F32R, tag=f"wprojr{cc}")
            nc.vector.tensor_copy(wr[:], w[:])
            Wproj_r.append(wr)

    xT = [persist.tile([128, N], F32R, tag=f"xT{cc}", name=f"xT{cc}") for cc in range(CC)]
    qT = [persist.tile([128, N], F32R, tag=f"qT{cc}", name=f"qT{cc}") for cc in range(CC)]
    v_pad = [persist.tile([128, H * 128], F32R, tag=f"vp{p}", name=f"vp{p}")
             for p in range(2)]
    qn = [persist.tile([128, 2 * 128], F32R, tag=f"qn{h}", name=f"qn{h}") for h in range(H)]
    G_sb = [persist.tile([128, 128], F32R, tag=f"G{h}", name=f"G{h}") for h in range(H)]
    Pz = [persist.tile([128, 128], F32R, tag=f"Pz{h}", name=f"Pz{h}") for h in range(H)]
    qg_sb = [persist.tile([128, N], F32R, tag=f"qg{h}", name=f"qg{h}") for h in range(H)]
    m_sb = persist.tile([128, H * N], F32, tag="m_sb")
    zT = [persist.tile([128, N], F32R, tag=f"zT{cc}", name=f"zT{cc}") for cc in range(CC)]
    xp_sb = [persist.tile([128, N], F32R, tag=f"xp{cc}", name=f"xp{cc}") for cc in range(CC)]
    sq_sb = [persist.tile([128, N], F32R, tag=f"sq{cc}", name=f"sq{cc}") for cc in range(CC)]

    # zero-fill pads once (via ACT so f32r consumers see a rounding producer)
    for h in range(H):
        par = h % 2
        for pc in range(2):
            nc.scalar.activation(
                v_pad[pc][:, h * 128 + (1 - par) * 64: h * 128 + (2 - par) * 64],
                zsrc[:, 0:64], AF.Copy, scale=0.0)
        for nch in range(2):
            nc.scalar.activation(
                qn[h][:, nch * 128 + (1 - par) * 64: nch * 128 + (2 - par) * 64],
                zsrc[:, 0:64], AF.Copy, scale=0.0)
        nc.scalar.activation(G_sb[h][:], zsrc[:, 0:128], AF.Copy, scale=0.0)
        nc.scalar.activation(Pz[h][:, (1 - par) * 64:(2 - par) * 64],
                             zsrc[:, 0:64], AF.Copy, scale=0.0)

    def one_pass(it):
        # ---- x -> xT (c-major) ----
        with tc.tile_pool(name=f"xload{it}", bufs=2) as xload, \
             tc.tile_pool(name=f"tps{it}", bufs=3, space="PSUM") as tps:
            for nch in range(2):
                xn = xload.tile([128, D], F32, tag="xn")
                nc.sync.dma_start(out=xn, in_=t_x.ap()[nch * 128:(nch + 1) * 128, :])
                for cc in range(CC):
                    pt = tps.tile([128, 128], F32, tag="pt")
                    nc.tensor.transpose(pt[:], xn[:, cc * 128:(cc + 1) * 128], ident[:])
                    nc.vector.tensor_copy(xT[cc][:, nch * 128:(nch + 1) * 128], pt[:])

        # ---- Gbar_h = s^2/R * Wk_h^T (ref^T ref) Wk_h ----
        # S = ref^T ref contracts over ref's natural partition dim: NO
        # transposes.  S accumulated in two PSUM passes (8 + 4 banks) over
        # streamed ref chunks; then U = S @ Wk, Gbar_h = Wk_h^T U_h via the
        # sliding head-pair lhsT trick.
        with tc.tile_pool(name=f"wkload{it}", bufs=3) as wkload, \
             tc.tile_pool(name=f"wkr{it}", bufs=1) as wkrp, \
             tc.tile_pool(name=f"refload{it}", bufs=4) as refload, \
             tc.tile_pool(name=f"ssb{it}", bufs=1) as ssbp, \
             tc.tile_pool(name=f"usb{it}", bufs=1) as usbp:
            Wk_r = []
            for cc in range(CC):
                w = wkload.tile([128, D], F32, tag="wkl")
                nc.sync.dma_start(out=w, in_=t_Wk.ap()[cc * 128:(cc + 1) * 128, :])
                wr = wkrp.tile([128, D], F32R, tag=f"wkr{cc}")
                nc.vector.tensor_copy(wr[:], w[:])
                Wk_r.append(wr)
            S_sb = [ssbp.tile([128, D], F32R, tag=f"S{cc}", name=f"S{it}_{cc}")
                    for cc in range(CC)]
            for p, cc1s in ((0, (0, 1, 2, 3)), (1, (4, 5))):
                with tc.tile_pool(name=f"sps0{it}_{p}", bufs=1, space="PSUM") as sp:
                    ps = {}
                    for cc1 in cc1s:
                        for jh in range(2):
                            ps[(cc1, jh)] = sp.tile(
                                [128, 384], F32, tag=f"ps{cc1}_{jh}",
                                name=f"ps{it}_{p}_{cc1}_{jh}")
                    for mch in range(2 * R):
                        rl = refload.tile([128, D], F32, tag="rl")
                        nc.sync.dma_start(
                            out=rl, in_=t_ref.ap()[mch * 128:(mch + 1) * 128, :])
                        rlr = refload.tile([128, D], F32R, tag="rlr")
                        if mch % 2 == 0:
                            nc.vector.tensor_copy(rlr[:], rl[:])
                        else:
                            nc.scalar.copy(rlr[:], rl[:])
                        for cc1 in cc1s:
                            for jh in range(2):
                                nc.tensor.matmul(
                                    ps[(cc1, jh)][:],
                                    rlr[:, cc1 * 128:(cc1 + 1) * 128],
                                    rlr[:, jh * 384:(jh + 1) * 384],
                                    start=(mch == 0), stop=(mch == 2 * R - 1))
                    for cc1 in cc1s:
                        for jh in range(2):
                            if (cc1 + jh) % 2 == 0:
                                nc.vector.tensor_copy(
                                    S_sb[cc1][:, jh * 384:(jh + 1) * 384], ps[(cc1, jh)][:])
                            else:
                                nc.scalar.copy(
                                    S_sb[cc1][:, jh * 384:(jh + 1) * 384], ps[(cc1, jh)][:])
            # U = S @ Wk (c1-part, j-free), zero-padded to 832 cols
            ups_ctx = ExitStack()
            ups = ups_ctx.enter_context(
                tc.tile_pool(name=f"ups{it}", bufs=3, space="PSUM"))
            gps = ups_ctx.enter_context(
                tc.tile_pool(name=f"gps{it}", bufs=2, space="PSUM"))
            U_sb = [usbp.tile([128, D + HD], F32R, tag=f"U{cc}", name=f"U{it}_{cc}")
                    for cc in range(CC)]
            for cc1 in range(CC):
                nc.scalar.activation(U_sb[cc1][:, D:D + HD], zsrc[:, 0:64],
                                     AF.Copy, scale=0.0)
                for jh in range(2):
                    pu = ups.tile([128, 384], F32, tag="pu")
                    for kc in range(CC):
                        nc.tensor.matmul(
                            pu[:], S_sb[kc][:, cc1 * 128:(cc1 + 1) * 128],
                            Wk_r[kc][:, jh * 384:(jh + 1) * 384],
                            start=(kc == 0), stop=(kc == CC - 1))
                    if jh == 0:
                        nc.vector.tensor_copy(U_sb[cc1][:, 0:384], pu[:])
                    else:
                        nc.scalar.copy(U_sb[cc1][:, 384:768], pu[:])
            # Gbar_h = Wk_h^T U_h: pair-block lhsT puts head h's Gram block
            # on rows par*64..; drain to the par-diagonal block of G_sb
            for h in range(H):
                pair, par = h // 2, h % 2
                pg = gps.tile([128, HD], F32, tag="pg", name=f"pg{it}_{h}")
                for kc in range(CC):
                    nc.tensor.matmul(
                        pg[:], Wk_r[kc][:, pair * 128: (pair + 1) * 128],
                        U_sb[kc][:, h * 64: h * 64 + 64],
                        start=(kc == 0), stop=(kc == CC - 1))
                nc.scalar.activation(
                    G_sb[h][par * 64:(par + 1) * 64, par * 64:(par + 1) * 64],
                    pg[par * 64:(par + 1) * 64, :], AF.Copy, scale=GS)
            ups_ctx.close()

        wqv_ctx = ExitStack()
        wqvres = wqv_ctx.enter_context(tc.tile_pool(name=f"wqvres{it}", bufs=1))
        wqv_stage = wqv_ctx.enter_context(tc.tile_pool(name=f"wqvstage{it}", bufs=3))
        Wqv_r = []
        for kc in range(CC):
            wl = wqv_stage.tile([128, 2 * D], F32, tag="wqvl", name=f"wqvl{it}_{kc}")
            nc.sync.dma_start(out=wl, in_=t_Wqv.ap()[kc * 128:(kc + 1) * 128, :])
            wr = wqvres.tile([128, 2 * D], F32R, tag=f"wqvr{kc}", name=f"wqvr{it}_{kc}")
            nc.vector.tensor_copy(wr[:], wl[:])
            Wqv_r.append(wr)
        for step in range(STEPS):
            # ---- A: qv^T = Wqv^T @ x^T ----
            with tc.tile_pool(name=f"qvps{it}_{step}", bufs=1, space="PSUM") as qvps, \
                 tc.tile_pool(name=f"vtps{it}_{step}", bufs=2, space="PSUM") as vtps, \
                 tc.tile_pool(name=f"vtmp{it}_{step}", bufs=2) as vtmp:
                for half in range(2):
                    pqv = [qvps.tile([128, N], F32, tag=f"pqv{j}",
                                     name=f"pqv{it}_{step}_{half}_{j}") for j in range(CC)]
                    for kc in range(CC):
                        for j in range(CC):
                            nc.tensor.matmul(
                                pqv[j][:],
                                Wqv_r[kc][:, half * D + j * 128: half * D + (j + 1) * 128],
                                xT[kc][:],
                                start=(kc == 0), stop=(kc == CC - 1))
                    for j in range(CC):
                        if half == 0:
                            nc.scalar.copy(qT[j][:], pqv[j][:])
                        else:
                            vt = vtmp.tile([128, N], F32, tag="vt")
                            nc.scalar.copy(vt[:], pqv[j][:])
                            for nch in range(2):
                                pt = vtps.tile([128, 128], F32, tag="vpt")
                                nc.tensor.transpose(pt[:], vt[:, nch * 128:(nch + 1) * 128],
                                                    ident[:])
                                nc.vector.tensor_copy(
                                    v_pad[nch][:, (2 * j) * 128 + 0:(2 * j) * 128 + 64],
                                    pt[:, 0:64])
                                nc.vector.tensor_copy(
                                    v_pad[nch][:, (2 * j + 1) * 128 + 64:(2 * j + 1) * 128 + 128],
                                    pt[:, 64:128])

            # ---- attention: q-nat transposes, w, P, z ----
            with tc.tile_pool(name=f"zps{it}_{step}", bufs=1, space="PSUM") as zps, \
                 tc.tile_pool(name=f"sps2{it}_{step}", bufs=1, space="PSUM") as sps2, \
                 tc.tile_pool(name=f"wps{it}_{step}", bufs=2, space="PSUM") as wps:
                zpsum = zps.tile([128, H * 128], F32, tag="z",
                                 name=f"zpsum{it}_{step}")  # 3 banks
                zbank_start = {}
                # q natural: one full-tile transpose per (pair, nch); the two
                # heads come out side by side in the free dim
                for pair in range(CC):
                    for nch in range(2):
                        pt = sps2.tile([128, 128], F32, tag="qnt")
                        nc.tensor.transpose(
                            pt[:], qT[pair][:, nch * 128:(nch + 1) * 128].bitcast(F32),
                            ident[:])
                        nc.vector.tensor_copy(
                            qn[2 * pair][:, nch * 128: nch * 128 + 64], pt[:, 0:64])
                        nc.vector.tensor_copy(
                            qn[2 * pair + 1][:, nch * 128 + 64: nch * 128 + 128],
                            pt[:, 64:128])
                for h in range(H):
                    pair, par = h // 2, h % 2
                    # qg^T = Gbar q^T (needs only A output; runs parallel to w)
                    pqg = wps.tile([128, N], F32, tag="qg", name=f"pqg{it}_{step}_{h}")
                    nc.tensor.matmul(pqg[:], G_sb[h][:], qT[pair][:],
                                     start=True, stop=True)
                    if par == 0:
                        nc.vector.tensor_copy(qg_sb[h][:], pqg[:])
                    else:
                        nc.scalar.copy(qg_sb[h][:], pqg[:])
                    # w = q^T v  (rows par*64.. via qn col-parity placement)
                    pw = wps.tile([128, HD], F32, tag="w", name=f"pw{it}_{step}_{h}")
                    for nch in range(2):
                        nc.tensor.matmul(
                            pw[:], qn[h][:, nch * 128:(nch + 1) * 128],
                            v_pad[nch][:, h * 128 + par * 64: h * 128 + (par + 1) * 64],
                            start=(nch == 0), stop=(nch == 1))
                    nc.scalar.copy(Pz[h][:, par * 64:(par + 1) * 64], pw[:])
                    # z^T[pair] += w^T-as-lhsT @ qg^T (par-packed output rows)
                    bank = pair // 2
                    is_start = (par == 0 and pair % 2 == 0)
                    mi = nc.tensor.matmul(
                        zpsum[:, pair * 256:(pair + 1) * 256],
                        Pz[h][:], qg_sb[h][:],
                        start=is_start,
                        stop=(par == 1 and pair % 2 == 1),
                        skip_group_check=True)
                    if is_start:
                        zbank_start[bank] = mi.ins
                    elif par == 0 and pair % 2 == 1:
                        add_dep_helper(mi.ins, zbank_start[bank], sync=False,
                                       reason="z region first-write after bank start")

                # ---- E: drain z + duplicate halves + strided regather ----
                for h in range(H):
                    par = h % 2
                    dst = m_sb[par * 64:(par + 1) * 64, h * N:(h + 1) * N]
                    src = zpsum[par * 64:(par + 1) * 64, (h // 2) * 256:(h // 2) * 256 + N]
                    if par == 0:
                        nc.vector.tensor_copy(dst, src)
                    else:
                        nc.scalar.copy(dst, src)
                ev = m_sb[0:64, :].rearrange("p (h n) -> p h n", n=N)[:, 0::2, :]
                ev_d = m_sb[64:128, :].rearrange("p (h n) -> p h n", n=N)[:, 0::2, :]
                od = m_sb[64:128, :].rearrange("p (h n) -> p h n", n=N)[:, 1::2, :]
                od_d = m_sb[0:64, :].rearrange("p (h n) -> p h n", n=N)[:, 1::2, :]
                nc.sync.dma_start(out=ev_d, in_=ev)
                nc.sync.dma_start(out=od_d, in_=od)
                for cc in range(CC):
                    nc.vector.tensor_copy(zT[cc][0:64, :], m_sb[0:64, 2 * cc::12])
                    nc.scalar.copy(zT[cc][64:128, :], m_sb[64:128, 2 * cc + 1::12])

            # ---- F: xp^T = Wproj^T @ z^T (+bproj);  G: LayerNorm over c ----
            with tc.tile_pool(name=f"fps{it}_{step}", bufs=2, space="PSUM") as fps, \
                 tc.tile_pool(name=f"sps{it}_{step}", bufs=2, space="PSUM") as sps, \
                 tc.tile_pool(name=f"ln{it}_{step}", bufs=1) as ln:
                for mc in range(CC):
                    pxp = fps.tile([128, N], F32, tag="pxp")
                    for kc in range(CC):
                        nc.tensor.matmul(
                            pxp[:], Wproj_r[kc][:, mc * 128:(mc + 1) * 128], zT[kc][:],
                            start=(kc == 0), stop=(kc == CC - 1))
                    nc.scalar.activation(
                        xp_sb[mc][:], pxp[:], AF.Identity, bias=bproj_sb[:, mc:mc + 1])
                for mc in range(CC):
                    nc.scalar.activation(sq_sb[mc][:], xp_sb[mc][:].bitcast(F32), AF.Square)
                psum_s = sps.tile([128, N], F32, tag="s", name=f"psum_s{it}_{step}")
                psum_q = sps.tile([128, N], F32, tag="q", name=f"psum_q{it}_{step}")
                for mc in range(CC):
                    nc.tensor.matmul(psum_s[:], ones128[:], xp_sb[mc][:],
                                     start=(mc == 0), stop=(mc == CC - 1))
                for mc in range(CC):
                    nc.tensor.matmul(psum_q[:], ones128[:], sq_sb[mc][:],
                                     start=(mc == 0), stop=(mc == CC - 1))
                mean_b = ln.tile([128, N], F32, tag="meanb")
                mean2_b = ln.tile([128, N], F32, tag="mean2b")
                var_b = ln.tile([128, N], F32, tag="varb")
                rsig_b = ln.tile([128, N], F32, tag="rsigb")
                nc.scalar.activation(mean_b[:], psum_s[:], AF.Copy, scale=1.0 / D)
                nc.vector.tensor_mul(mean2_b[:], mean_b[:], mean_b[:])
                nc.vector.scalar_tensor_tensor(
                    out=var_b[:], in0=psum_q[:], scalar=1.0 / D, in1=mean2_b[:],
                    op0=mybir.AluOpType.mult, op1=mybir.AluOpType.subtract)
                nc.scalar.activation(var_b[:], var_b[:], AF.Sqrt, bias=eps_sb[:])
                nc.vector.reciprocal(rsig_b[:], var_b[:])
                tmp = ln.tile([128, N], F32, tag="lntmp")
                for mc in range(CC):
                    nc.vector.tensor_sub(tmp[:], xp_sb[mc][:].bitcast(F32), mean_b[:])
                    nc.vector.scalar_tensor_tensor(
                        out=tmp[:], in0=tmp[:], scalar=gamma_sb[:, mc:mc + 1],
                        in1=rsig_b[:],
                        op0=mybir.AluOpType.mult, op1=mybir.AluOpType.mult)
                    nc.vector.tensor_scalar_add(
                        out=xT[mc][:], in0=tmp[:], scalar1=beta_sb[:, mc:mc + 1])

        wqv_ctx.close()

        # ---- epilogue: transpose x^T -> x, store ----
        with tc.tile_pool(name=f"eps{it}", bufs=3, space="PSUM") as eps_pool, \
             tc.tile_pool(name=f"osb{it}", bufs=1) as osb:
            out_nat = [osb.tile([128, D], F32, tag=f"on{it}_{nch}",
                                name=f"on{it}_{nch}") for nch in range(2)]
            for cc in range(CC):
                for nch in range(2):
                    pt = eps_pool.tile([128, 128], F32, tag="ept")
                    nc.tensor.transpose(
                        pt[:], xT[cc][:, nch * 128:(nch + 1) * 128].bitcast(F32), ident[:])
                    nc.vector.tensor_copy(out_nat[nch][:, cc * 128:(cc + 1) * 128], pt[:])
            for nch in range(2):
                nc.sync.dma_start(out=t_out.ap()[nch * 128:(nch + 1) * 128, :],
                                  in_=out_nat[nch][:])

    if iters == 1:
        one_pass(0)
    else:
        with tc.For_i(0, iters, 1):
            one_pass(0)


def build(iters=1):
    nc = bacc.Bacc("TRN2", target_bir_lowering=False, debug=False, num_devices=NB)
    t_x = nc.declare_dram_parameter("x", [N, D], F32, isOutput=False)
    t_ref = nc.declare_dram_parameter("ref", [R * N, D], F32, isOutput=False)
    t_Wqv = nc.declare_dram_parameter("Wqv", [D, 2 * D], F32, isOutput=False)
    t_Wk = nc.declare_dram_parameter("Wk", [D, D], F32, isOutput=False)
    t_Wproj = nc.declare_dram_parameter("Wproj", [D, D], F32, isOutput=False)
    t_bproj = nc.declare_dram_parameter("bproj", [D], F32, isOutput=False)
    t_gamma = nc.declare_dram_parameter("gamma", [D], F32, isOutput=False)
    t_beta = nc.declare_dram_parameter("beta", [D], F32, isOutput=False)
    t_out = nc.declare_dram_parameter("out", [N, D], F32, isOutput=True)
    with tile.TileContext(nc) as tc:
        with ExitStack() as ctx:
            _emit(nc, tc, ctx, t_x, t_ref, t_Wqv, t_Wk, t_Wproj, t_bproj,
                  t_gamma, t_beta, t_out, iters=iters)
    nc.compile()
    return nc


_CACHE = {}
last_results = None


def kernel(x, ref, Wqv, Wk, Wproj, bproj, gamma, beta):
    global last_results
    if "nc" not in _CACHE:
        _CACHE["nc"] = build()
    nc = _CACHE["nc"]

    def f(a):
        return np.ascontiguousarray(np.asarray(a), dtype=np.float32)

    x = f(x)
    common = dict(ref=f(ref).reshape(R * N, D), Wqv=f(Wqv), Wk=f(Wk),
                  Wproj=f(Wproj), bproj=f(bproj), gamma=f(gamma), beta=f(beta))
    in_maps = [dict(x=x[b], **common) for b in range(NB)]
    res = run_bass_kernel_spmd(nc, in_maps, list(range(NB)))
    last_results = res
    return np.stack([res.results[b]["out"] for b in range(NB)]).astype(np.float32)
